# revision 10
# baseline (speedup 1.0000x reference)
"""Trainium2 Bass kernel for CycleBalanceLoss.

loss = ALPHA * mean_b |sum_l adj[b, argmax_l, argmax_{l+1}]|
     + (1-ALPHA) * mean_{b,l} (logsumexp(logits[b,l,:]) - logits[b,l,t[b,l]])

Sharding: pure data parallel over the batch dim B=64 across 8 cores
(BPC=8 batches per core). Host sums the 8 per-core partial scalars.

Per core:
  - stream the logits shard [8, 128, 1024] f32 through SBUF;
  - ScalarE computes exp(x) with a per-row f32 accumulator (-> logsumexp)
    writing the exp values as fp16: argmax(exp(x)) == argmax(x), so the
    DVE max/max_index pass then runs on 2-byte data;
  - instead of per-element indirect DMAs (the old bottleneck: 16 serialized
    DMA_INDIRECTs ~2us each on GpSimd), both gathers use the Pool SWDGE
    dma_gather, which fetches the aligned 256B/512B block CONTAINING each
    needed element in bulk (994ns + 0.34ns/descriptor):
      * target logits: one dma_gather of 1024 blocks (64 f32 each), block
        indices fully host-precomputed (wrapped int16 layout);
      * adjacency weights: block indices depend on the on-device argmax;
        they are assembled in natural [l, b] layout, then routed into the
        SWDGE wrapped-int16 layout with a PE permute matmul + a DRAM
        round-trip (DRAM APs allow the partition<->free reshuffle) + a PE
        replicate matmul; two dma_gathers of 512 blocks (128 f32 each)
        cover batches 0-3 / 4-7 (int16 block-index range limit);
  - the within-block element select is (iota == rem) * blocks summed along
    the block axis with plain DVE ops (tensor_mask_reduce wedges this HW).
"""

import numpy as np

B, L, N = 64, 128, 1024
NCORES = 8
BPC = B // NCORES
ALPHA = 0.7

XE = 64    # xt gather block elems (256B)
AE = 128   # adj gather block elems (512B)

_CACHE = {}


def _build():
    import concourse.bacc as bacc
    import concourse.tile as tile
    from concourse import bass, library_config, mybir
    from concourse.bass import broadcast_tensor_aps

    f32 = mybir.dt.float32
    fp16 = mybir.dt.float16
    i16 = mybir.dt.int16
    u16 = mybir.dt.uint16
    AF = mybir.ActivationFunctionType
    Alu = mybir.AluOpType
    AX = mybir.AxisListType

    nc = bacc.Bacc(
        "TRN2",
        target_bir_lowering=False,
        debug=False,
        num_devices=NCORES,
    )

    # logits shard as [16384, 64]: row r = flat elems [64r, 64r+64)
    logits = nc.dram_tensor("logits", [BPC * L * N // XE, XE], f32, kind="ExternalInput")
    adj = nc.dram_tensor("adj", [BPC * N * N // AE, AE], f32, kind="ExternalInput")
    xtidx = nc.dram_tensor("xtidx", [128, 64], i16, kind="ExternalInput")
    xtrem = nc.dram_tensor("xtrem", [128, BPC], f32, kind="ExternalInput")
    adjcc = nc.dram_tensor("adjcc", [128, BPC], u16, kind="ExternalInput")
    kmat = nc.dram_tensor("kmat", [128, 8], f32, kind="ExternalInput")
    w16 = nc.dram_tensor("w16", [128, 128], f32, kind="ExternalInput")
    iota = nc.dram_tensor("iota", [128, AE], f32, kind="ExternalInput")
    out = nc.dram_tensor("out", [2, 1], f32, kind="ExternalOutput")

    lg = logits.ap()

    with tile.TileContext(nc) as tc:
        with (
            tc.tile_pool(name="xp", bufs=4) as xp,
            tc.tile_pool(name="ep", bufs=3) as ep,
            tc.tile_pool(name="sp", bufs=2) as sp,
            tc.tile_pool(name="acc", bufs=1) as accp,
            tc.tile_pool(name="psum", bufs=2, space="PSUM") as pp,
        ):
            nc.gpsimd.load_library(library_config.mlp)

            ones = accp.tile([L, 1], f32)
            nc.vector.memset(ones[:], 1.0)

            # constants / host-precomputed tables
            XTI = accp.tile([128, 64], i16)
            nc.sync.dma_start(XTI[:], xtidx.ap())
            XR = accp.tile([128, BPC], f32)
            nc.sync.dma_start(XR[:], xtrem.ap())
            CC = accp.tile([128, BPC], u16)
            nc.sync.dma_start(CC[:], adjcc.ap())
            KM = accp.tile([128, 8], f32)
            nc.sync.dma_start(KM[:], kmat.ap())
            WM = accp.tile([128, 128], f32)
            nc.sync.dma_start(WM[:], w16.ap())
            IO = accp.tile([128, AE], f32)
            nc.sync.dma_start(IO[:], iota.ap())

            # target-logit blocks: gather can start immediately (host idxs)
            XTB = accp.tile([128, BPC, XE], f32)
            nc.gpsimd.dma_gather(
                XTB[:], lg, XTI[:], BPC * 128, BPC * 128, XE
            )

            S = accp.tile([L, BPC], f32)    # per-(l,b) sum of exp
            IDX = accp.tile([L, BPC * 8], u16)  # max_index outputs, col0 of 8 per batch
            M8 = accp.tile([L, BPC * 8], fp16)

            for b in range(BPC):
                X = xp.tile([L, N], f32, tag="X")
                src = lg[b * 2048 : (b + 1) * 2048].rearrange(
                    "(l s) e -> l s e", l=L, s=N // XE
                )
                if b % 2 == 0:
                    nc.sync.dma_start(X[:].rearrange("l (s e) -> l s e", s=N // XE), src)
                else:
                    nc.scalar.dma_start(X[:].rearrange("l (s e) -> l s e", s=N // XE), src)

                E = ep.tile([L, N], fp16, tag="E")
                nc.scalar.activation(E[:], X[:], AF.Exp, accum_out=S[:, b : b + 1])
                nc.vector.max(M8[:, 8 * b : 8 * b + 8], E[:])
                nc.vector.max_index(
                    IDX[:, 8 * b : 8 * b + 8], M8[:, 8 * b : 8 * b + 8], E[:]
                )

            # ---- adjacency block indices (natural [l, b] layout) ----
            idxv = IDX[:].rearrange("p (b e) -> p b e", e=8)[:, :, 0:1]
            srcv = idxv.rearrange("p b e -> p (b e)")  # [128, BPC] u16 strided
            dsh = accp.tile([L, BPC], u16)
            nc.vector.memset(dsh[:], 0)
            nc.scalar.dma_start(
                dsh[0 : L - 1, :],
                IDX[1:L].rearrange("p (b e) -> p b e", e=8)[:, :, 0:1].rearrange(
                    "p b e -> p (b e)"
                ),
            )

            t1 = sp.tile([L, BPC], u16, tag="t1")
            nc.vector.tensor_scalar(t1[:], srcv, 3, None, op0=Alu.logical_shift_left)
            t2 = sp.tile([L, BPC], u16, tag="t2")
            nc.vector.tensor_scalar(t2[:], dsh[:], 7, None, op0=Alu.logical_shift_right)
            blk = sp.tile([L, BPC], u16, tag="blk")
            nc.vector.tensor_tensor(blk[:], t1[:], t2[:], op=Alu.add)
            nc.vector.tensor_tensor(blk[:], blk[:], CC[:], op=Alu.add)

            remu = sp.tile([L, BPC], u16, tag="remu")
            nc.vector.tensor_scalar(remu[:], dsh[:], AE - 1, None, op0=Alu.bitwise_and)
            remf = accp.tile([L, BPC], f32)
            nc.vector.tensor_copy(remf[:], remu[:])

            # ---- wrap blk into the SWDGE int16 idx layout ----
            # rhs2[l, 8b+j] = blk[l, b] * K[l, j];  K[l, j] = (l//16 == j)
            # out[q, 8b+j] = sum_l (l%16 == q%16) * rhs2[l, 8b+j]
            #             = blk[16j + q%16, b]  -> wrapped + replicated.
            blkf = sp.tile([L, BPC], f32, tag="blkf")
            nc.vector.tensor_copy(blkf[:], blk[:])
            rhs2 = sp.tile([128, BPC, 8], f32, tag="rhs2")
            b1, b2 = broadcast_tensor_aps(
                blkf[:].rearrange("p (b u) -> p b u", u=1),
                KM[:].rearrange("p (u j) -> p u j", u=1),
            )
            nc.vector.tensor_tensor(rhs2[:], b1, b2, op=Alu.mult)
            m2 = pp.tile([128, 64], f32)
            nc.tensor.matmul(
                out=m2[:],
                lhsT=WM[:],
                rhs=rhs2[:].rearrange("p b j -> p (b j)"),
                start=True,
                stop=True,
            )
            AIDX = accp.tile([128, 64], i16)
            nc.vector.tensor_copy(AIDX[:], m2[:])

            # ---- adjacency block gathers (batches 0-3 / 4-7) ----
            av = adj.ap()
            half = BPC * N * N // AE // 2
            ADJB = accp.tile([128, BPC, AE], f32)
            nc.gpsimd.dma_gather(
                ADJB[:, 0 : BPC // 2, :], av[0:half], AIDX[:, 0:32], 512, 512, AE
            )
            nc.gpsimd.dma_gather(
                ADJB[:, BPC // 2 : BPC, :], av[half : 2 * half], AIDX[:, 32:64], 512, 512, AE
            )

            # ---- within-block selects: (iota == rem) * blocks, sum ----
            iox = IO[:, 0:XE].rearrange("p (u e) -> p u e", u=1)
            xrb = XR[:].rearrange("p (b u) -> p b u", u=1)
            a1, a2 = broadcast_tensor_aps(iox, xrb)
            eqx = sp.tile([128, BPC, XE], f32, tag="eqx")
            nc.vector.tensor_tensor(eqx[:], a1, a2, op=Alu.is_equal)
            nc.vector.tensor_tensor(eqx[:], eqx[:], XTB[:], op=Alu.mult)
            XTv = accp.tile([128, BPC], f32)
            nc.vector.tensor_reduce(XTv[:], eqx[:], axis=AX.X, op=Alu.add)

            ioa = IO[:].rearrange("p (u e) -> p u e", u=1)
            arb = remf[:].rearrange("p (b u) -> p b u", u=1)
            a3, a4 = broadcast_tensor_aps(ioa, arb)
            eqa = sp.tile([128, BPC, AE], f32, tag="eqa")
            nc.vector.tensor_tensor(eqa[:], a3, a4, op=Alu.is_equal)
            nc.vector.tensor_tensor(eqa[:], eqa[:], ADJB[:], op=Alu.mult)
            Wv = accp.tile([128, BPC], f32)
            nc.vector.tensor_reduce(Wv[:], eqa[:], axis=AX.X, op=Alu.add)

            # ---- cross-entropy partial: sum(ln S - x_t) ----
            LSE = accp.tile([L, BPC], f32)
            nc.scalar.activation(LSE[:], S[:], AF.Ln)
            R = accp.tile([L, 2], f32)
            nc.vector.memset(R[:, 1:2], 0.0)
            NLL = accp.tile([L, BPC], f32)
            nc.vector.tensor_sub(NLL[:], LSE[:], XTv[:])
            nc.vector.reduce_sum(R[:, 0:1], NLL[:], axis=AX.X)

            # ---- balance partial: sum_b |sum_l W| (row 127 of Wv is pad) ----
            ps_b = pp.tile([BPC, 1], f32)
            nc.tensor.matmul(
                out=ps_b[:], lhsT=Wv[0 : L - 1, :], rhs=ones[0 : L - 1, :],
                start=True, stop=True,
            )
            bneg = sp.tile([BPC, 1], f32, tag="bneg")
            nc.vector.tensor_scalar_mul(bneg[:], ps_b[:], -1.0)
            nc.vector.tensor_tensor(R[0:BPC, 1:2], ps_b[:], bneg[:], op=Alu.max)

            ps2 = pp.tile([2, 1], f32)
            nc.tensor.matmul(out=ps2[:], lhsT=R[:], rhs=ones[:], start=True, stop=True)
            c2 = sp.tile([2, 1], f32, tag="c2")
            nc.vector.tensor_copy(c2[:], ps2[:])
            nc.scalar.dma_start(out.ap(), c2[:])

    nc.compile()
    return nc


def _get_nc():
    if "nc" not in _CACHE:
        _CACHE["nc"] = _build()
    return _CACHE["nc"]


def _consts():
    if "consts" in _CACHE:
        return _CACHE["consts"]
    ls = np.arange(128)
    kmatm = (ls[:, None] // 16 == np.arange(8)[None, :]).astype(np.float32)
    w16m = (ls[:, None] % 16 == ls[None, :] % 16).astype(np.float32)
    iotam = np.tile(np.arange(AE, dtype=np.float32), (128, 1))
    adjccm = np.tile(
        ((np.arange(BPC) % 4) * (N * N // AE)).astype(np.uint16), (128, 1)
    )
    _CACHE["consts"] = (kmatm, w16m, iotam, adjccm)
    return _CACHE["consts"]


def make_in_maps(path_logits, target_paths, adj_matrix):
    """Shard full inputs into per-core in_maps (host-side packing only)."""
    kmatm, w16m, iotam, adjccm = _consts()
    l_arange = np.arange(L, dtype=np.int64)
    in_maps = []
    for c in range(NCORES):
        sl = slice(c * BPC, (c + 1) * BPC)
        lgc = np.ascontiguousarray(path_logits[sl], dtype=np.float32).reshape(
            BPC * L * N // XE, XE
        )
        adc = np.ascontiguousarray(adj_matrix[sl], dtype=np.float32).reshape(
            BPC * N * N // AE, AE
        )
        t = np.asarray(target_paths[sl], dtype=np.int64)  # [BPC, L]
        # xt block idx for (b, l): b*2048 + l*16 + (t >> 6), wrapped:
        # g = b*128 + l -> tile[(g%16) + 16r, g//16]
        blkx = (
            np.arange(BPC, dtype=np.int64)[:, None] * (L * N // XE)
            + l_arange[None, :] * (N // XE)
            + (t >> 6)
        )  # [BPC, L]
        xti = np.zeros((16, 64), np.int16)
        g = (np.arange(BPC)[:, None] * 128 + l_arange[None, :]).ravel()
        xti[g % 16, g // 16] = blkx.ravel()
        xti = np.tile(xti, (8, 1))
        xtr = np.ascontiguousarray((t & (XE - 1)).T.astype(np.float32))  # [L, BPC]
        in_maps.append(
            {
                "logits": lgc,
                "adj": adc,
                "xtidx": xti,
                "xtrem": xtr,
                "adjcc": adjccm,
                "kmat": kmatm,
                "w16": w16m,
                "iota": iotam,
            }
        )
    return in_maps


def kernel(**inputs):
    from concourse import bass_utils

    nc = _get_nc()
    in_maps = make_in_maps(
        inputs["path_logits"], inputs["target_paths"], inputs["adj_matrix"]
    )
    res = bass_utils.run_bass_kernel_spmd(nc, in_maps, core_ids=list(range(NCORES)))
    w_nll = np.float32((1.0 - ALPHA) / (B * L))
    w_bal = np.float32(ALPHA / B)
    total = np.float32(0.0)
    for r in res.results:
        total = total + w_nll * np.float32(r["out"][0, 0]) + w_bal * np.float32(
            r["out"][1, 0]
        )
    return np.asarray(total, dtype=np.float32)


# revision 13
# speedup vs baseline: 1.3252x; 1.3252x over previous
"""Trainium2 Bass kernel for CycleBalanceLoss.

loss = ALPHA * mean_b |sum_l adj[b, argmax_l, argmax_{l+1}]|
     + (1-ALPHA) * mean_{b,l} (logsumexp(logits[b,l,:]) - logits[b,l,t[b,l]])

Sharding: pure data parallel over the batch dim B=64 across 8 cores
(BPC=8 batches per core). Host sums the 8 per-core partial scalars.

Per core:
  - stream the logits shard [8, 128, 1024] f32 through SBUF;
  - ScalarE computes exp(x) with a per-row f32 accumulator (-> logsumexp)
    writing the exp values as fp16: argmax(exp(x)) == argmax(x), so the
    DVE max/max_index pass runs on 2-byte data;
  - both gathers use the Pool SWDGE dma_gather, which fetches the aligned
    256B/512B block CONTAINING each needed element in bulk:
      * target logits: one dma_gather of 1024 blocks (64 f32), indices
        fully host-precomputed in the wrapped int16 layout;
      * adjacency: block indices depend on the on-device argmax; built in
        natural [l, b] layout, then folded into the wrapped layout with
        one broadcast-multiply + one PE matmul (W16REP does the
        partition fold AND the 8x replication); two dma_gathers of 512
        blocks (128 f32) cover batches 0-3 / 4-7 (int16 range limit),
        with the first group's chain overlapping the second half of the
        batch loop;
  - within-block select: (iota == rem) * blocks summed along the block
    axis with plain DVE ops (tensor_mask_reduce wedges this HW path);
  - tc.tile_wait_until phases pin the queue order: the tile scheduler's
    SWDGE cost model is optimistic, so without the phases it hoists
    gather-dependent selects ahead of the per-batch argmax work, stalling
    the in-order Vector queue on the gather.
"""

import numpy as np

B, L, N = 64, 128, 1024
NCORES = 8
BPC = B // NCORES
ALPHA = 0.7

XE = 64    # xt gather block elems (256B)
AE = 128   # adj gather block elems (512B)

_CACHE = {}


def _build():
    import concourse.bacc as bacc
    import concourse.tile as tile
    from concourse import bass, library_config, mybir
    from concourse.bass import broadcast_tensor_aps

    f32 = mybir.dt.float32
    fp16 = mybir.dt.float16
    i16 = mybir.dt.int16
    u16 = mybir.dt.uint16
    AF = mybir.ActivationFunctionType
    Alu = mybir.AluOpType
    AX = mybir.AxisListType

    nc = bacc.Bacc(
        "TRN2",
        target_bir_lowering=False,
        debug=False,
        num_devices=NCORES,
    )

    logits = nc.dram_tensor("logits", [BPC * L * N // XE, XE], f32, kind="ExternalInput")
    adj = nc.dram_tensor("adj", [BPC * N * N // AE, AE], f32, kind="ExternalInput")
    # cf: [0:8 xtrem | 8:16 kmat | 16:144 w16 | 144:272 iota]
    cf = nc.dram_tensor("cf", [128, 272], f32, kind="ExternalInput")
    # cu: [0:64 xtidx (wrapped int16) | 64:72 adjcc]
    cu = nc.dram_tensor("cu", [128, 72], i16, kind="ExternalInput")
    out = nc.dram_tensor("out", [2, 1], f32, kind="ExternalOutput")

    lg = logits.ap()
    av = adj.ap()
    GB = 4            # batches per adj gather group
    NGRP = BPC // GB  # 2
    half = BPC * N * N // AE // NGRP

    with tile.TileContext(nc) as tc:
        with (
            tc.tile_pool(name="xp", bufs=4) as xp,
            tc.tile_pool(name="ep", bufs=3) as ep,
            tc.tile_pool(name="sp", bufs=2) as sp,
            tc.tile_pool(name="acc", bufs=1) as accp,
            tc.tile_pool(name="psum", bufs=2, space="PSUM") as pp,
        ):
            nc.gpsimd.load_library(library_config.mlp)

            CU = accp.tile([128, 72], i16)
            nc.sync.dma_start(CU[:], cu.ap())
            CF = accp.tile([128, 272], f32)
            nc.sync.dma_start(CF[:], cf.ap())
            XR = CF[:, 0:BPC]
            KM = CF[:, 8:16]
            WM = CF[:, 16:144]
            IO = CF[:, 144 : 144 + AE]
            CC = CU[:, 64:72].bitcast(u16)

            ones = accp.tile([L, 1], f32)
            nc.vector.memset(ones[:], 1.0)
            dsh = accp.tile([L, BPC], u16)
            nc.vector.memset(dsh[:], 0)

            # target-logit blocks: gather can start immediately (host idxs)
            XTB = accp.tile([128, BPC, XE], f32)
            nc.gpsimd.dma_gather(XTB[:], lg, CU[:, 0:64], BPC * 128, BPC * 128, XE)

            S = accp.tile([L, BPC], f32)
            IDX = accp.tile([L, BPC * 8], u16)
            M8 = accp.tile([L, BPC * 8], fp16)
            ADJB = accp.tile([128, BPC, AE], f32)
            AIDX = accp.tile([128, 64], i16)
            Wv = accp.tile([128, BPC], f32)
            remf = accp.tile([L, BPC], f32)

            def batch(b):
                X = xp.tile([L, N], f32, tag="X")
                src = lg[b * 2048 : (b + 1) * 2048].rearrange(
                    "(l s) e -> l s e", l=L, s=N // XE
                )
                eng = nc.sync if b % 2 == 0 else nc.scalar
                eng.dma_start(X[:].rearrange("l (s e) -> l s e", s=N // XE), src)
                E = ep.tile([L, N], fp16, tag="E")
                nc.scalar.activation(E[:], X[:], AF.Exp, accum_out=S[:, b : b + 1])
                nc.vector.max(M8[:, 8 * b : 8 * b + 8], E[:])
                nc.vector.max_index(
                    IDX[:, 8 * b : 8 * b + 8], M8[:, 8 * b : 8 * b + 8], E[:]
                )

            def idxcols(lo, hi, shift=0):
                # strided view of IDX col 0 per batch, batches [lo, hi)
                base = IDX[shift:L] if shift else IDX[:]
                return base.rearrange("p (b e) -> p b e", e=8)[
                    :, lo:hi, 0:1
                ].rearrange("p b e -> p (b e)")

            def adj_group(g):
                lo, hi = g * GB, (g + 1) * GB
                cols = slice(lo, hi)
                nc.scalar.dma_start(dsh[0 : L - 1, cols], idxcols(lo, hi, shift=1))
                t2 = sp.tile([L, GB], u16, tag=f"t2{g}")
                nc.vector.tensor_scalar(
                    t2[:], dsh[:, cols], 7, None, op0=Alu.logical_shift_right
                )
                nc.vector.tensor_tensor(t2[:], t2[:], CC[:, cols], op=Alu.add)
                blk = sp.tile([L, GB], u16, tag=f"blk{g}")
                nc.vector.tensor_scalar(
                    blk[:], idxcols(lo, hi), 3, None, op0=Alu.logical_shift_left
                )
                nc.vector.tensor_tensor(blk[:], blk[:], t2[:], op=Alu.add)
                blkf = sp.tile([L, GB], f32, tag=f"blkf{g}")
                nc.vector.tensor_copy(blkf[:], blk[:])
                rhs2 = sp.tile([128, GB, 8], f32, tag=f"rhs2{g}")
                b1, b2 = broadcast_tensor_aps(
                    blkf[:].rearrange("p (b u) -> p b u", u=1),
                    KM.rearrange("p (u j) -> p u j", u=1),
                )
                nc.vector.tensor_tensor(rhs2[:], b1, b2, op=Alu.mult)
                m2 = pp.tile([128, GB * 8], f32)
                nc.tensor.matmul(
                    out=m2[:], lhsT=WM,
                    rhs=rhs2[:].rearrange("p b j -> p (b j)"),
                    start=True, stop=True,
                )
                nc.vector.tensor_copy(AIDX[:, 32 * g : 32 * g + 32], m2[:])
                nc.gpsimd.dma_gather(
                    ADJB[:, lo:hi, :],
                    av[g * half : (g + 1) * half],
                    AIDX[:, 32 * g : 32 * g + 32],
                    GB * 128, GB * 128, AE,
                )
                # rem for the in-block select (not gather-critical)
                remu = sp.tile([L, GB], u16, tag=f"remu{g}")
                nc.vector.tensor_scalar(
                    remu[:], dsh[:, cols], AE - 1, None, op0=Alu.bitwise_and
                )
                nc.vector.tensor_copy(remf[:, cols], remu[:])

            for b in range(GB):
                with tc.tile_wait_until(0.002 * b):
                    batch(b)
            with tc.tile_wait_until(0.009):
                adj_group(0)
            for b in range(GB, BPC):
                with tc.tile_wait_until(0.002 * b):
                    batch(b)
            with tc.tile_wait_until(0.017):
                adj_group(1)

            with tc.tile_wait_until(0.0175):
                # XT in-block select (runs in the shadow of the adj gather)
                iox = IO[:, 0:XE].rearrange("p (u e) -> p u e", u=1)
                xrb = XR.rearrange("p (b u) -> p b u", u=1)
                a1, a2 = broadcast_tensor_aps(iox, xrb)
                eqx = sp.tile([128, BPC, XE], f32, tag="eqx")
                nc.vector.tensor_tensor(eqx[:], a1, a2, op=Alu.is_equal)
                nc.vector.tensor_tensor(eqx[:], eqx[:], XTB[:], op=Alu.mult)
                XTv = accp.tile([128, BPC], f32)
                nc.vector.tensor_reduce(XTv[:], eqx[:], axis=AX.X, op=Alu.add)

                # cross-entropy partial
                LSE = accp.tile([L, BPC], f32)
                nc.scalar.activation(LSE[:], S[:], AF.Ln)
                R = accp.tile([L, 2], f32)
                nc.vector.memset(R[:, 1:2], 0.0)
                NLL = accp.tile([L, BPC], f32)
                nc.vector.tensor_sub(NLL[:], LSE[:], XTv[:])
                nc.vector.reduce_sum(R[:, 0:1], NLL[:], axis=AX.X)

            with tc.tile_wait_until(0.019):
                # adj in-block select
                ioa = IO.rearrange("p (u e) -> p u e", u=1)
                arb = remf[:].rearrange("p (b u) -> p b u", u=1)
                a3, a4 = broadcast_tensor_aps(ioa, arb)
                eqa = sp.tile([128, BPC, AE], f32, tag="eqa")
                nc.vector.tensor_tensor(eqa[:], a3, a4, op=Alu.is_equal)
                nc.vector.tensor_tensor(eqa[:], eqa[:], ADJB[:], op=Alu.mult)
                nc.vector.tensor_reduce(Wv[:], eqa[:], axis=AX.X, op=Alu.add)

                # balance partial: row 127 of Wv is pad
                ps_b = pp.tile([BPC, 1], f32)
                nc.tensor.matmul(
                    out=ps_b[:], lhsT=Wv[0 : L - 1, :], rhs=ones[0 : L - 1, :],
                    start=True, stop=True,
                )
                bneg = sp.tile([BPC, 1], f32, tag="bneg")
                nc.vector.tensor_scalar_mul(bneg[:], ps_b[:], -1.0)
                nc.vector.tensor_tensor(R[0:BPC, 1:2], ps_b[:], bneg[:], op=Alu.max)

                ps2 = pp.tile([2, 1], f32)
                nc.tensor.matmul(out=ps2[:], lhsT=R[:], rhs=ones[:], start=True, stop=True)
                c2 = sp.tile([2, 1], f32, tag="c2")
                nc.vector.tensor_copy(c2[:], ps2[:])
                nc.scalar.dma_start(out.ap(), c2[:])

    nc.compile()
    return nc


def _get_nc():
    if "nc" not in _CACHE:
        _CACHE["nc"] = _build()
    return _CACHE["nc"]


def _consts():
    if "consts" in _CACHE:
        return _CACHE["consts"]
    ls = np.arange(128)
    cfm = np.zeros((128, 272), np.float32)
    cfm[:, 8:16] = (ls[:, None] // 16 == np.arange(8)[None, :]).astype(np.float32)
    cfm[:, 16:144] = (ls[:, None] % 16 == ls[None, :] % 16).astype(np.float32)
    cfm[:, 144 : 144 + AE] = np.arange(AE, dtype=np.float32)[None, :]
    adjccm = ((np.arange(BPC) % GB_HOST) * (N * N // AE)).astype(np.int16)
    _CACHE["consts"] = (cfm, adjccm)
    return _CACHE["consts"]


GB_HOST = 4  # batches per adj gather group (must match _build's GB)


def make_in_maps(path_logits, target_paths, adj_matrix):
    """Shard full inputs into per-core in_maps (host-side packing only)."""
    cfm, adjccm = _consts()
    l_arange = np.arange(L, dtype=np.int64)
    in_maps = []
    for c in range(NCORES):
        sl = slice(c * BPC, (c + 1) * BPC)
        lgc = np.ascontiguousarray(path_logits[sl], dtype=np.float32).reshape(
            BPC * L * N // XE, XE
        )
        adc = np.ascontiguousarray(adj_matrix[sl], dtype=np.float32).reshape(
            BPC * N * N // AE, AE
        )
        t = np.asarray(target_paths[sl], dtype=np.int64)  # [BPC, L]
        blkx = (
            np.arange(BPC, dtype=np.int64)[:, None] * (L * N // XE)
            + l_arange[None, :] * (N // XE)
            + (t >> 6)
        )
        xti = np.zeros((16, 64), np.int16)
        g = (np.arange(BPC)[:, None] * 128 + l_arange[None, :]).ravel()
        xti[g % 16, g // 16] = blkx.ravel()
        cum = np.zeros((128, 72), np.int16)
        cum[:, 0:64] = np.tile(xti, (8, 1))
        cum[:, 64:72] = adjccm[None, :]
        cfc = cfm.copy()
        cfc[:, 0:BPC] = (t & (XE - 1)).T.astype(np.float32)
        in_maps.append({"logits": lgc, "adj": adc, "cf": cfc, "cu": cum})
    return in_maps


def kernel(**inputs):
    from concourse import bass_utils

    nc = _get_nc()
    in_maps = make_in_maps(
        inputs["path_logits"], inputs["target_paths"], inputs["adj_matrix"]
    )
    res = bass_utils.run_bass_kernel_spmd(nc, in_maps, core_ids=list(range(NCORES)))
    w_nll = np.float32((1.0 - ALPHA) / (B * L))
    w_bal = np.float32(ALPHA / B)
    total = np.float32(0.0)
    for r in res.results:
        total = total + w_nll * np.float32(r["out"][0, 0]) + w_bal * np.float32(
            r["out"][1, 0]
        )
    return np.asarray(total, dtype=np.float32)


# revision 15
# speedup vs baseline: 1.3303x; 1.0039x over previous
"""Trainium2 Bass kernel for CycleBalanceLoss.

loss = ALPHA * mean_b |sum_l adj[b, argmax_l, argmax_{l+1}]|
     + (1-ALPHA) * mean_{b,l} (logsumexp(logits[b,l,:]) - logits[b,l,t[b,l]])

Sharding: pure data parallel over the batch dim B=64 across 8 cores
(BPC=8 batches per core). Host sums the 8 per-core partial scalars.

Per core:
  - stream the logits shard [8, 128, 1024] f32 through SBUF;
  - ScalarE computes exp(x) with a per-row f32 accumulator (-> logsumexp)
    writing exp as fp16 so the DVE argmax (max/max_index) runs on 2-byte
    data (argmax(exp(x)) == argmax(x));
  - both gathers use Pool SWDGE dma_gather fetching the aligned 256B/512B
    block CONTAINING each needed element (vs. the old 16 serialized
    per-element DMA_INDIRECTs):
      * target logits: one dma_gather, indices host-precomputed in the
        wrapped int16 layout;
      * adjacency: indices depend on the device argmax. The idx[l+1]
        partition shift is a PE matmul with a shift matrix (zeroes the
        pad row for free); the wrapped-int16 fold+replicate is one
        broadcast-multiply + one PE matmul (W16REP). Three gather groups
        [0-3], [4-6], [7] (int16 block-index range caps a group at 4
        batches) so earlier groups overlap the batch loop and the tail
        only carries a 128-descriptor gather;
  - within-block selects are single fused scalar_tensor_tensor ops:
    (iota == rem[p]) * blocks with a sum accumulator;
  - tc.tile_wait_until phases pin queue order: the scheduler's SWDGE cost
    model is optimistic and otherwise hoists gather-dependent ops ahead
    of the argmax work, stalling the in-order DVE queue.
"""

import numpy as np

B, L, N = 64, 128, 1024
NCORES = 8
BPC = B // NCORES
ALPHA = 0.7

XE = 64    # xt gather block elems (256B)
AE = 128   # adj gather block elems (512B)
GROUPS = [(0, 4), (4, 7), (7, 8)]  # adj gather groups [lo, hi)

_CACHE = {}


def _build():
    import concourse.bacc as bacc
    import concourse.tile as tile
    from concourse import bass, library_config, mybir
    from concourse.bass import broadcast_tensor_aps

    f32 = mybir.dt.float32
    fp16 = mybir.dt.float16
    i16 = mybir.dt.int16
    u16 = mybir.dt.uint16
    AF = mybir.ActivationFunctionType
    Alu = mybir.AluOpType
    AX = mybir.AxisListType

    nc = bacc.Bacc(
        "TRN2",
        target_bir_lowering=False,
        debug=False,
        num_devices=NCORES,
    )

    logits = nc.dram_tensor("logits", [BPC * L * N // XE, XE], f32, kind="ExternalInput")
    adj = nc.dram_tensor("adj", [BPC * N * N // AE, AE], f32, kind="ExternalInput")
    # cf: [0:8 xtrem | 8:16 kmat | 16:144 w16 | 144:272 iota | 272:400 shiftm | 400:408 ccf]
    cf = nc.dram_tensor("cf", [128, 408], f32, kind="ExternalInput")
    # cu: wrapped int16 xt block idxs
    cu = nc.dram_tensor("cu", [128, 64], i16, kind="ExternalInput")
    out = nc.dram_tensor("out", [2, 1], f32, kind="ExternalOutput")

    lg = logits.ap()
    av = adj.ap()
    ROWS_PER_B = N * N // AE  # adj view rows per batch

    with tile.TileContext(nc) as tc:
        with (
            tc.tile_pool(name="xp", bufs=4) as xp,
            tc.tile_pool(name="ep", bufs=3) as ep,
            tc.tile_pool(name="sp", bufs=2) as sp,
            tc.tile_pool(name="acc", bufs=1) as accp,
            tc.tile_pool(name="psum", bufs=1, space="PSUM") as pp,
        ):
            nc.gpsimd.load_library(library_config.mlp)

            CU = accp.tile([128, 64], i16)
            nc.scalar.dma_start(CU[:], cu.ap())
            CF = accp.tile([128, 408], f32)
            nc.scalar.dma_start(CF[:], cf.ap())
            XR = CF[:, 0:BPC]
            KM = CF[:, 8:16]
            WM = CF[:, 16:144]
            IO = CF[:, 144 : 144 + AE]
            SH = CF[:, 272:400]
            CCF = CF[:, 400:408]

            ones = accp.tile([L, 1], f32)
            nc.vector.memset(ones[:], 1.0)

            # target-logit blocks: gather starts as soon as CU lands
            XTB = accp.tile([128, BPC, XE], f32)
            nc.gpsimd.dma_gather(XTB[:], lg, CU[:], BPC * 128, BPC * 128, XE)

            S = accp.tile([L, BPC], f32)
            IDX = accp.tile([L, BPC * 8], u16)
            M8 = accp.tile([L, BPC * 8], fp16)
            ADJB = accp.tile([128, BPC, AE], f32)
            AIDX = accp.tile([128, 64], i16)
            Wv = accp.tile([128, BPC], f32)
            XTv = accp.tile([128, BPC], f32)
            remf = accp.tile([L, BPC], f32)

            def batch(b):
                X = xp.tile([L, N], f32, tag="X")
                src = lg[b * 2048 : (b + 1) * 2048].rearrange(
                    "(l s) e -> l s e", l=L, s=N // XE
                )
                nc.sync.dma_start(X[:].rearrange("l (s e) -> l s e", s=N // XE), src)
                E = ep.tile([L, N], fp16, tag="E")
                nc.scalar.activation(E[:], X[:], AF.Exp, accum_out=S[:, b : b + 1])
                nc.vector.max(M8[:, 8 * b : 8 * b + 8], E[:])
                nc.vector.max_index(
                    IDX[:, 8 * b : 8 * b + 8], M8[:, 8 * b : 8 * b + 8], E[:]
                )

            def idxcols(lo, hi):
                return IDX[:].rearrange("p (b e) -> p b e", e=8)[
                    :, lo:hi, 0:1
                ].rearrange("p b e -> p (b e)")

            def adj_group(g):
                lo, hi = GROUPS[g]
                G = hi - lo
                cols = slice(lo, hi)
                # hi/lo parts of idx (natural layout), as f32 for the PE shift
                hl_u = sp.tile([L, 2 * G], u16, tag=f"hlu{g}")
                nc.vector.tensor_scalar(
                    hl_u[:, 0:G], idxcols(lo, hi), 7, None,
                    op0=Alu.logical_shift_right,
                )
                nc.vector.tensor_scalar(
                    hl_u[:, G : 2 * G], idxcols(lo, hi), AE - 1, None,
                    op0=Alu.bitwise_and,
                )
                hl_f = sp.tile([L, 2 * G], f32, tag=f"hlf{g}")
                nc.vector.tensor_copy(hl_f[:], hl_u[:])
                srcf = sp.tile([L, G], f32, tag=f"srcf{g}")
                nc.vector.tensor_copy(srcf[:], idxcols(lo, hi))
                # partition shift l -> l+1 via PE (row 127 becomes 0)
                shp = pp.tile([L, 2 * G], f32)
                nc.tensor.matmul(out=shp[:], lhsT=SH, rhs=hl_f[:], start=True, stop=True)
                nc.vector.tensor_copy(remf[:, cols], shp[:, G : 2 * G])
                # blk = src*8 + shifted_hi + cc
                blkf = sp.tile([L, G], f32, tag=f"blkf{g}")
                nc.vector.scalar_tensor_tensor(
                    blkf[:], srcf[:], 8.0, shp[:, 0:G], op0=Alu.mult, op1=Alu.add
                )
                nc.vector.tensor_tensor(blkf[:], blkf[:], CCF[:, cols], op=Alu.add)
                # fold into wrapped layout: rhs2 = blk (x) K, m2 = W16REP^T @ rhs2
                rhs2 = sp.tile([128, G, 8], f32, tag=f"rhs2{g}")
                b1, b2 = broadcast_tensor_aps(
                    blkf[:].rearrange("p (b u) -> p b u", u=1),
                    KM.rearrange("p (u j) -> p u j", u=1),
                )
                nc.vector.tensor_tensor(rhs2[:], b1, b2, op=Alu.mult)
                m2 = pp.tile([128, G * 8], f32)
                nc.tensor.matmul(
                    out=m2[:], lhsT=WM, rhs=rhs2[:].rearrange("p b j -> p (b j)"),
                    start=True, stop=True,
                )
                nc.vector.tensor_copy(AIDX[:, 8 * lo : 8 * hi], m2[:])
                nc.gpsimd.dma_gather(
                    ADJB[:, cols, :],
                    av[lo * ROWS_PER_B : hi * ROWS_PER_B],
                    AIDX[:, 8 * lo : 8 * hi],
                    G * 128, G * 128, AE,
                )
                # fused select: Wv[:, b] = sum_k (iota==rem) * block
                scrA = sp.tile([128, AE], f32, tag=f"scrA{g}")
                for b in range(lo, hi):
                    nc.vector.scalar_tensor_tensor(
                        scrA[:], IO, remf[:, b : b + 1], ADJB[:, b, :],
                        op0=Alu.is_equal, op1=Alu.mult,
                        accum_out=Wv[:, b : b + 1],
                    )

            for b in range(4):
                with tc.tile_wait_until(0.002 * b):
                    batch(b)
            with tc.tile_wait_until(0.0085):
                adj_group(0)
            for b in range(4, BPC):
                with tc.tile_wait_until(0.002 * b):
                    batch(b)

            with tc.tile_wait_until(0.0125):
                # fused XT selects (XTB ready long before; fills DVE gaps)
                scrX = sp.tile([128, XE], f32, tag="scrX")
                for b in range(BPC):
                    nc.vector.scalar_tensor_tensor(
                        scrX[:], IO[:, 0:XE], XR[:, b : b + 1], XTB[:, b, :],
                        op0=Alu.is_equal, op1=Alu.mult,
                        accum_out=XTv[:, b : b + 1],
                    )

            with tc.tile_wait_until(0.0145):
                adj_group(1)
            with tc.tile_wait_until(0.016):
                adj_group(2)

            with tc.tile_wait_until(0.0165):
                # cross-entropy partial
                LSE = accp.tile([L, BPC], f32)
                nc.scalar.activation(LSE[:], S[:], AF.Ln)
                R = accp.tile([L, 2], f32)
                nc.vector.memset(R[:, 1:2], 0.0)
                NLL = accp.tile([L, BPC], f32)
                nc.vector.tensor_sub(NLL[:], LSE[:], XTv[:])
                nc.vector.reduce_sum(R[:, 0:1], NLL[:], axis=AX.X)

            with tc.tile_wait_until(0.018):
                # balance partial: row 127 of Wv is pad
                ps_b = pp.tile([BPC, 1], f32)
                nc.tensor.matmul(
                    out=ps_b[:], lhsT=Wv[0 : L - 1, :], rhs=ones[0 : L - 1, :],
                    start=True, stop=True,
                )
                bneg = sp.tile([BPC, 1], f32, tag="bneg")
                nc.vector.tensor_scalar_mul(bneg[:], ps_b[:], -1.0)
                nc.vector.tensor_tensor(R[0:BPC, 1:2], ps_b[:], bneg[:], op=Alu.max)

                ps2 = pp.tile([2, 1], f32)
                nc.tensor.matmul(out=ps2[:], lhsT=R[:], rhs=ones[:], start=True, stop=True)
                c2 = sp.tile([2, 1], f32, tag="c2")
                nc.vector.tensor_copy(c2[:], ps2[:])
                nc.scalar.dma_start(out.ap(), c2[:])

    nc.compile()
    return nc


def _get_nc():
    if "nc" not in _CACHE:
        _CACHE["nc"] = _build()
    return _CACHE["nc"]


def _consts():
    if "consts" in _CACHE:
        return _CACHE["consts"]
    ls = np.arange(128)
    cfm = np.zeros((128, 408), np.float32)
    cfm[:, 8:16] = (ls[:, None] // 16 == np.arange(8)[None, :]).astype(np.float32)
    cfm[:, 16:144] = (ls[:, None] % 16 == ls[None, :] % 16).astype(np.float32)
    cfm[:, 144 : 144 + AE] = np.arange(AE, dtype=np.float32)[None, :]
    cfm[:, 272:400] = (ls[:, None] == ls[None, :] + 1).astype(np.float32)
    cc = np.zeros(BPC, np.float32)
    for lo, hi in GROUPS:
        cc[lo:hi] = (np.arange(hi - lo)) * (N * N // AE)
    cfm[:, 400:408] = cc[None, :]
    _CACHE["consts"] = cfm
    return _CACHE["consts"]


def make_in_maps(path_logits, target_paths, adj_matrix):
    """Shard full inputs into per-core in_maps (host-side packing only)."""
    cfm = _consts()
    l_arange = np.arange(L, dtype=np.int64)
    in_maps = []
    for c in range(NCORES):
        sl = slice(c * BPC, (c + 1) * BPC)
        lgc = np.ascontiguousarray(path_logits[sl], dtype=np.float32).reshape(
            BPC * L * N // XE, XE
        )
        adc = np.ascontiguousarray(adj_matrix[sl], dtype=np.float32).reshape(
            BPC * N * N // AE, AE
        )
        t = np.asarray(target_paths[sl], dtype=np.int64)  # [BPC, L]
        blkx = (
            np.arange(BPC, dtype=np.int64)[:, None] * (L * N // XE)
            + l_arange[None, :] * (N // XE)
            + (t >> 6)
        )
        xti = np.zeros((16, 64), np.int16)
        g = (np.arange(BPC)[:, None] * 128 + l_arange[None, :]).ravel()
        xti[g % 16, g // 16] = blkx.ravel()
        cum = np.tile(xti, (8, 1))
        cfc = cfm.copy()
        cfc[:, 0:BPC] = (t & (XE - 1)).T.astype(np.float32)
        in_maps.append({"logits": lgc, "adj": adc, "cf": cfc, "cu": cum})
    return in_maps


def kernel(**inputs):
    from concourse import bass_utils

    nc = _get_nc()
    in_maps = make_in_maps(
        inputs["path_logits"], inputs["target_paths"], inputs["adj_matrix"]
    )
    res = bass_utils.run_bass_kernel_spmd(nc, in_maps, core_ids=list(range(NCORES)))
    w_nll = np.float32((1.0 - ALPHA) / (B * L))
    w_bal = np.float32(ALPHA / B)
    total = np.float32(0.0)
    for r in res.results:
        total = total + w_nll * np.float32(r["out"][0, 0]) + w_bal * np.float32(
            r["out"][1, 0]
        )
    return np.asarray(total, dtype=np.float32)


# revision 18
# speedup vs baseline: 1.3587x; 1.0213x over previous
"""Trainium2 Bass kernel for CycleBalanceLoss.

loss = ALPHA * mean_b |sum_l adj[b, argmax_l, argmax_{l+1}]|
     + (1-ALPHA) * mean_{b,l} (logsumexp(logits[b,l,:]) - logits[b,l,t[b,l]])

Sharding: pure data parallel over the batch dim B=64 across 8 cores
(BPC=8 batches per core). Host sums the 8 per-core partial scalars.

Per core:
  - stream the logits shard [8, 128, 1024] f32 through SBUF;
  - ScalarE computes exp(x) with a per-row f32 accumulator (-> logsumexp)
    writing exp as fp16 so the DVE argmax (max/max_index) runs on 2-byte
    data (argmax(exp(x)) == argmax(x));
  - both gathers use Pool SWDGE dma_gather fetching the aligned 256B/512B
    block CONTAINING each needed element (vs. the old 16 serialized
    per-element DMA_INDIRECTs):
      * target logits: one dma_gather, indices host-precomputed in the
        wrapped int16 layout;
      * adjacency: indices depend on the device argmax. The idx[l+1]
        partition shift is a PE matmul with a shift matrix (zeroes the
        pad row for free); the wrapped-int16 fold+replicate is one
        broadcast-multiply + one PE matmul (W16REP). Three gather groups
        [0-3], [4-6], [7] (int16 block-index range caps a group at 4
        batches) so earlier groups overlap the batch loop and the tail
        only carries a 128-descriptor gather;
  - within-block selects are single fused scalar_tensor_tensor ops:
    (iota == rem[p]) * blocks with a sum accumulator;
  - tc.tile_wait_until phases pin queue order: the scheduler's SWDGE cost
    model is optimistic and otherwise hoists gather-dependent ops ahead
    of the argmax work, stalling the in-order DVE queue.
"""

import numpy as np

B, L, N = 64, 128, 1024
NCORES = 8
BPC = B // NCORES
ALPHA = 0.7

XE = 64    # xt gather block elems (256B)
AE = 128   # adj gather block elems (512B)
GROUPS = [(0, 4), (4, 7), (7, 8)]  # adj gather groups [lo, hi)

_CACHE = {}


def _build():
    import concourse.bacc as bacc
    import concourse.tile as tile
    from concourse import bass, library_config, mybir
    from concourse.bass import broadcast_tensor_aps

    f32 = mybir.dt.float32
    fp16 = mybir.dt.float16
    i16 = mybir.dt.int16
    u16 = mybir.dt.uint16
    AF = mybir.ActivationFunctionType
    Alu = mybir.AluOpType
    AX = mybir.AxisListType

    nc = bacc.Bacc(
        "TRN2",
        target_bir_lowering=False,
        debug=False,
        num_devices=NCORES,
    )

    logits = nc.dram_tensor("logits", [BPC * L * N // XE, XE], f32, kind="ExternalInput")
    adj = nc.dram_tensor("adj", [BPC * N * N // AE, AE], f32, kind="ExternalInput")
    # cf: [0:8 xtrem | 8:16 kmat | 16:144 w16 | 144:272 iota | 272:400 shiftm | 400:408 ccf]
    cf = nc.dram_tensor("cf", [128, 408], f32, kind="ExternalInput")
    # cu: wrapped int16 xt block idxs
    cu = nc.dram_tensor("cu", [128, 64], i16, kind="ExternalInput")
    out = nc.dram_tensor("out", [2, 1], f32, kind="ExternalOutput")

    lg = logits.ap()
    av = adj.ap()
    ROWS_PER_B = N * N // AE  # adj view rows per batch

    with tile.TileContext(nc) as tc:
        with (
            tc.tile_pool(name="xp", bufs=4) as xp,
            tc.tile_pool(name="ep", bufs=3) as ep,
            tc.tile_pool(name="sp", bufs=2) as sp,
            tc.tile_pool(name="acc", bufs=1) as accp,
            tc.tile_pool(name="psum", bufs=1, space="PSUM") as pp,
        ):
            nc.gpsimd.load_library(library_config.mlp)

            CU = accp.tile([128, 64], i16)
            nc.scalar.dma_start(CU[:], cu.ap())
            CF = accp.tile([128, 408], f32)
            nc.scalar.dma_start(CF[:], cf.ap())
            XR = CF[:, 0:BPC]
            KM = CF[:, 8:16]
            WM = CF[:, 16:144]
            IO = CF[:, 144 : 144 + AE]
            SH = CF[:, 272:400]
            CCF = CF[:, 400:408]

            ones = accp.tile([L, 1], f32)
            nc.vector.memset(ones[:], 1.0)

            # target-logit blocks: gather starts as soon as CU lands
            XTB = accp.tile([128, BPC, XE], f32)
            nc.gpsimd.dma_gather(XTB[:], lg, CU[:], BPC * 128, BPC * 128, XE)

            S = accp.tile([L, BPC], f32)
            IDXC = accp.tile([L, BPC], u16)  # argmax col per batch, contiguous
            M8 = accp.tile([L, BPC * 8], fp16)
            ADJB = accp.tile([128, BPC, AE], f32)
            AIDX = accp.tile([128, 64], i16)
            Wv = accp.tile([128, BPC], f32)
            XTv = accp.tile([128, BPC], f32)
            remf = accp.tile([L, BPC], f32)

            def batch(b):
                X = xp.tile([L, N], f32, tag="X")
                src = lg[b * 2048 : (b + 1) * 2048].rearrange(
                    "(l s) e -> l s e", l=L, s=N // XE
                )
                nc.sync.dma_start(X[:].rearrange("l (s e) -> l s e", s=N // XE), src)
                E = ep.tile([L, N], fp16, tag="E")
                nc.scalar.activation(E[:], X[:], AF.Exp, accum_out=S[:, b : b + 1])
                nc.vector.max(M8[:, 8 * b : 8 * b + 8], E[:])
                i8 = sp.tile([L, 8], u16, tag="i8")
                nc.vector.max_index(i8[:], M8[:, 8 * b : 8 * b + 8], E[:])
                nc.vector.tensor_copy(IDXC[:, b : b + 1], i8[:, 0:1])

            def idxcols(lo, hi):
                return IDXC[:, lo:hi]

            def adj_group(g):
                lo, hi = GROUPS[g]
                G = hi - lo
                cols = slice(lo, hi)
                # hi/lo parts of idx (natural layout), as f32 for the PE shift
                hl_u = sp.tile([L, 2 * G], u16, tag=f"hlu{g}")
                nc.vector.tensor_scalar(
                    hl_u[:, 0:G], idxcols(lo, hi), 7, None,
                    op0=Alu.logical_shift_right,
                )
                nc.vector.tensor_scalar(
                    hl_u[:, G : 2 * G], idxcols(lo, hi), AE - 1, None,
                    op0=Alu.bitwise_and,
                )
                hl_f = sp.tile([L, 2 * G], f32, tag=f"hlf{g}")
                nc.vector.tensor_copy(hl_f[:], hl_u[:])
                srcf = sp.tile([L, G], f32, tag=f"srcf{g}")
                nc.vector.tensor_copy(srcf[:], idxcols(lo, hi))
                # partition shift l -> l+1 via PE (row 127 becomes 0)
                shp = pp.tile([L, 2 * G], f32)
                nc.tensor.matmul(out=shp[:], lhsT=SH, rhs=hl_f[:], start=True, stop=True)
                nc.vector.tensor_copy(remf[:, cols], shp[:, G : 2 * G])
                # blk = src*8 + shifted_hi + cc
                blkf = sp.tile([L, G], f32, tag=f"blkf{g}")
                nc.vector.scalar_tensor_tensor(
                    blkf[:], srcf[:], 8.0, shp[:, 0:G], op0=Alu.mult, op1=Alu.add
                )
                nc.vector.tensor_tensor(blkf[:], blkf[:], CCF[:, cols], op=Alu.add)
                # fold into wrapped layout: rhs2 = blk (x) K, m2 = W16REP^T @ rhs2
                rhs2 = sp.tile([128, G, 8], f32, tag=f"rhs2{g}")
                b1, b2 = broadcast_tensor_aps(
                    blkf[:].rearrange("p (b u) -> p b u", u=1),
                    KM.rearrange("p (u j) -> p u j", u=1),
                )
                nc.vector.tensor_tensor(rhs2[:], b1, b2, op=Alu.mult)
                m2 = pp.tile([128, G * 8], f32)
                nc.tensor.matmul(
                    out=m2[:], lhsT=WM, rhs=rhs2[:].rearrange("p b j -> p (b j)"),
                    start=True, stop=True,
                )
                nc.vector.tensor_copy(AIDX[:, 8 * lo : 8 * hi], m2[:])
                nc.gpsimd.dma_gather(
                    ADJB[:, cols, :],
                    av[lo * ROWS_PER_B : hi * ROWS_PER_B],
                    AIDX[:, 8 * lo : 8 * hi],
                    G * 128, G * 128, AE,
                )
                # fused select: Wv[:, b] = sum_k (iota==rem) * block
                scrA = sp.tile([128, AE], f32, tag=f"scrA{g}")
                for b in range(lo, hi):
                    nc.vector.scalar_tensor_tensor(
                        scrA[:], IO, remf[:, b : b + 1], ADJB[:, b, :],
                        op0=Alu.is_equal, op1=Alu.mult,
                        accum_out=Wv[:, b : b + 1],
                    )

            for b in range(4):
                with tc.tile_wait_until(0.002 * b):
                    batch(b)
            with tc.tile_wait_until(0.0085):
                adj_group(0)
            for b in range(4, BPC):
                with tc.tile_wait_until(0.002 * b):
                    batch(b)

            with tc.tile_wait_until(0.0125):
                # fused XT selects (XTB ready long before; fills DVE gaps)
                scrX = sp.tile([128, XE], f32, tag="scrX")
                for b in range(BPC):
                    nc.vector.scalar_tensor_tensor(
                        scrX[:], IO[:, 0:XE], XR[:, b : b + 1], XTB[:, b, :],
                        op0=Alu.is_equal, op1=Alu.mult,
                        accum_out=XTv[:, b : b + 1],
                    )

            with tc.tile_wait_until(0.0133):
                adj_group(1)
            with tc.tile_wait_until(0.016):
                adj_group(2)

            with tc.tile_wait_until(0.0165):
                # cross-entropy partial
                LSE = accp.tile([L, BPC], f32)
                nc.scalar.activation(LSE[:], S[:], AF.Ln)
                R = accp.tile([L, 2], f32)
                nc.vector.memset(R[:, 1:2], 0.0)
                NLL = accp.tile([L, BPC], f32)
                nc.vector.tensor_sub(NLL[:], LSE[:], XTv[:])
                nc.vector.reduce_sum(R[:, 0:1], NLL[:], axis=AX.X)

            with tc.tile_wait_until(0.018):
                # balance partial: row 127 of Wv is pad
                ps_b = pp.tile([BPC, 1], f32)
                nc.tensor.matmul(
                    out=ps_b[:], lhsT=Wv[0 : L - 1, :], rhs=ones[0 : L - 1, :],
                    start=True, stop=True,
                )
                bneg = sp.tile([BPC, 1], f32, tag="bneg")
                nc.vector.tensor_scalar_mul(bneg[:], ps_b[:], -1.0)
                nc.vector.tensor_tensor(R[0:BPC, 1:2], ps_b[:], bneg[:], op=Alu.max)

                ps2 = pp.tile([2, 1], f32)
                nc.tensor.matmul(out=ps2[:], lhsT=R[:], rhs=ones[:], start=True, stop=True)
                c2 = sp.tile([2, 1], f32, tag="c2")
                nc.vector.tensor_copy(c2[:], ps2[:])
                nc.scalar.dma_start(out.ap(), c2[:])

    nc.compile()
    return nc


def _get_nc():
    if "nc" not in _CACHE:
        _CACHE["nc"] = _build()
    return _CACHE["nc"]


def _consts():
    if "consts" in _CACHE:
        return _CACHE["consts"]
    ls = np.arange(128)
    cfm = np.zeros((128, 408), np.float32)
    cfm[:, 8:16] = (ls[:, None] // 16 == np.arange(8)[None, :]).astype(np.float32)
    cfm[:, 16:144] = (ls[:, None] % 16 == ls[None, :] % 16).astype(np.float32)
    cfm[:, 144 : 144 + AE] = np.arange(AE, dtype=np.float32)[None, :]
    cfm[:, 272:400] = (ls[:, None] == ls[None, :] + 1).astype(np.float32)
    cc = np.zeros(BPC, np.float32)
    for lo, hi in GROUPS:
        cc[lo:hi] = (np.arange(hi - lo)) * (N * N // AE)
    cfm[:, 400:408] = cc[None, :]
    _CACHE["consts"] = cfm
    return _CACHE["consts"]


def make_in_maps(path_logits, target_paths, adj_matrix):
    """Shard full inputs into per-core in_maps (host-side packing only)."""
    cfm = _consts()
    l_arange = np.arange(L, dtype=np.int64)
    in_maps = []
    for c in range(NCORES):
        sl = slice(c * BPC, (c + 1) * BPC)
        lgc = np.ascontiguousarray(path_logits[sl], dtype=np.float32).reshape(
            BPC * L * N // XE, XE
        )
        adc = np.ascontiguousarray(adj_matrix[sl], dtype=np.float32).reshape(
            BPC * N * N // AE, AE
        )
        t = np.asarray(target_paths[sl], dtype=np.int64)  # [BPC, L]
        blkx = (
            np.arange(BPC, dtype=np.int64)[:, None] * (L * N // XE)
            + l_arange[None, :] * (N // XE)
            + (t >> 6)
        )
        xti = np.zeros((16, 64), np.int16)
        g = (np.arange(BPC)[:, None] * 128 + l_arange[None, :]).ravel()
        xti[g % 16, g // 16] = blkx.ravel()
        cum = np.tile(xti, (8, 1))
        cfc = cfm.copy()
        cfc[:, 0:BPC] = (t & (XE - 1)).T.astype(np.float32)
        in_maps.append({"logits": lgc, "adj": adc, "cf": cfc, "cu": cum})
    return in_maps


def kernel(**inputs):
    from concourse import bass_utils

    nc = _get_nc()
    in_maps = make_in_maps(
        inputs["path_logits"], inputs["target_paths"], inputs["adj_matrix"]
    )
    res = bass_utils.run_bass_kernel_spmd(nc, in_maps, core_ids=list(range(NCORES)))
    w_nll = np.float32((1.0 - ALPHA) / (B * L))
    w_bal = np.float32(ALPHA / B)
    total = np.float32(0.0)
    for r in res.results:
        total = total + w_nll * np.float32(r["out"][0, 0]) + w_bal * np.float32(
            r["out"][1, 0]
        )
    return np.asarray(total, dtype=np.float32)
